# revision 37
# baseline (speedup 1.0000x reference)
"""Multi-head attention (B=4, T=2048, C=1024, H=16, D=64) on 8 TRN2 cores.

Sharding: core i handles batch b=i//2 and the 8 heads of half hh=i%2.
Each core computes its heads' contribution through the row-sharded output
projection -> partial yT [C, T]; host transposes and sums the two partials
per batch.

v3: all matmul operands bf16 (fp32 "HIGH-mode" matmuls run at ~half PE rate
and block FastWeightLoad). The softmax exp on ScalarE (~157us) is the
per-pair bottleneck, so all PE work that is not on the exp critical path
(q/k projection quarters of the SAME pair one t-tile ahead, v projection
s-quarters, the deferred normalization of the PREVIOUS pair) is interleaved
into the attention j-loop where the PE otherwise idles waiting for exp.
Output projection emits yT = wpt_chunk.T @ outcat so the bias add becomes a
per-partition scalar on the (idle by then) ScalarE.

Per-core layouts (host pre-arranged, bf16):
  xT  [C, T]    = x[b].T
  wq/wk/wv [C, 512]  columns = (local head)*64 + d
  wpt [512, C]  rows  = (local head)*64 + d   (= Wp.T row-slice)
  bp  [C] f32   bias on even cores, zeros on odd (summed partials)
Output: yT [C, T] f32 (host transposes).

On-chip dataflow per core:
  qT/kT [128, T] per head-pair via lhsT=w-chunk, rhs=xT-chunk (N=512)
  v     [s, h, d] natural via lhsT=xT s-slice, rhs=wv (N=512, all 8 heads)
  scoresT[s,t]: lhsT=kT s-block [64,128], rhs=qT t-tile [64,<=512],
                2 heads row-tiled (K=64 each, concurrent on the PE array)
  exp on ScalarE PSUM->SBUF bf16 with scale=1/sqrt(C); causal via additive
  -1e9 mask on the straddling 128-blocks
  PV: lhsT=[v ; ones] [128,65] bf16, rhs=pT -> outT [65,512] PSUM per head,
  accumulated over s-blocks; row 64 = softmax normalizer Z
  normalize: reciprocal_approx_fast(Z) -> bf16 -> per-row K=1 matmul
  broadcast -> DVE mult into outcat (deferred one pair)
  yT: lhsT=wpt c'-chunk, rhs=outcat t-chunk; bias via ScalarE Identity
"""

import os
import sys

import numpy as np

for _p in ("/opt/trn_rl_repo", "/root/.axon_site/_ro/trn_rl_repo"):
    if os.path.isdir(_p) and _p not in sys.path:
        sys.path.append(_p)

import concourse.bass as bass
import concourse.bacc as bacc
import concourse.mybir as mybir
import concourse.tile as tile
from concourse.bass_utils import run_bass_kernel_spmd

B, T, C, H, D = 4, 2048, 1024, 16, 64
HL = H // 2          # heads per core
P = 128
NCH = C // P         # 8 c-chunks
NTT = T // 512       # 4 t-tiles of 512
NSB = T // P         # 16 s-blocks of 128
SCALE = 1.0 / 32.0   # 1/sqrt(C)

F32 = mybir.dt.float32
BF16 = mybir.dt.bfloat16

# zb row (j,u) -> idx=2j+u at partition 32*(idx//3), slot idx%3
# (AP base partitions are restricted to {0,32,64})
def _zslot(j, u):
    idx = 2 * j + u
    return 32 * (idx // 3), idx % 3


def _build(causal: bool) -> bass.Bass:
    nc = bacc.Bacc("TRN2", target_bir_lowering=False, debug=False, num_devices=8)

    xT = nc.dram_tensor("xT", [C, T], BF16, kind="ExternalInput").ap()
    wq_d = nc.dram_tensor("wq", [C, HL * D], BF16, kind="ExternalInput").ap()
    wk_d = nc.dram_tensor("wk", [C, HL * D], BF16, kind="ExternalInput").ap()
    wv_d = nc.dram_tensor("wv", [C, HL * D], BF16, kind="ExternalInput").ap()
    wpt_d = nc.dram_tensor("wpt", [HL * D, C], BF16, kind="ExternalInput").ap()
    bp_d = nc.dram_tensor("bp", [C], F32, kind="ExternalInput").ap()
    y_d = nc.dram_tensor("y", [C, T], F32, kind="ExternalOutput").ap()

    with tile.TileContext(nc) as tc:
        _emit(nc, tc, causal, xT, wq_d, wk_d, wv_d, wpt_d, bp_d, y_d)
    nc.compile()
    return nc


def _emit(nc, tc, causal, xT, wq_d, wk_d, wv_d, wpt_d, bp_d, y_d):
    from contextlib import ExitStack

    ctx = ExitStack()
    with ctx:
        consts = ctx.enter_context(tc.tile_pool(name="consts", bufs=1))
        x_pool = ctx.enter_context(tc.tile_pool(name="xh", bufs=1))
        w_pool = ctx.enter_context(tc.tile_pool(name="w", bufs=1))
        q_pool = ctx.enter_context(tc.tile_pool(name="qT", bufs=3))
        k_pool = ctx.enter_context(tc.tile_pool(name="kT", bufs=3))
        v_pool = ctx.enter_context(tc.tile_pool(name="v", bufs=1))
        oc_pool = ctx.enter_context(tc.tile_pool(name="outcat", bufs=4))
        p_pool = ctx.enter_context(tc.tile_pool(name="pT", bufs=4))
        z_pool = ctx.enter_context(tc.tile_pool(name="zb", bufs=2))
        wpt_pool = ctx.enter_context(tc.tile_pool(name="wpt", bufs=4))
        bpc_pool = ctx.enter_context(tc.tile_pool(name="bpc", bufs=1))
        yst_pool = ctx.enter_context(tc.tile_pool(name="yst", bufs=3))
        bps_pool = ctx.enter_context(tc.tile_pool(name="bps", bufs=3))
        psA = ctx.enter_context(tc.tile_pool(name="psA", bufs=2, space="PSUM"))
        psO = ctx.enter_context(tc.tile_pool(name="psO", bufs=2, space="PSUM"))
        psQ = ctx.enter_context(tc.tile_pool(name="psQ", bufs=2, space="PSUM"))

        # ---- constants ----
        # additive causal mask: 0 where free>=partition else -1e9
        mask = None
        if causal:
            mask = consts.tile([P, 2, P], F32)
            nc.vector.memset(mask, 0.0)
            for _u in range(2):
                nc.gpsimd.affine_select(
                    out=mask[:, _u, :], in_=mask[:, _u, :],
                    compare_op=mybir.AluOpType.is_ge,
                    fill=-1e9, base=0,
                    pattern=[[1, P]], channel_multiplier=-1,
                )
        ones_bc = consts.tile([P, P], BF16)
        nc.vector.memset(ones_bc, 1.0)

        # ---- DMA: weights first (first matmuls need them), x in c-chunks ----
        wq_t = w_pool.tile([P, NCH, HL * D], BF16, tag="wq", name="wq")
        wk_t = w_pool.tile([P, NCH, HL * D], BF16, tag="wk", name="wk")
        wv_t = w_pool.tile([P, NCH, HL * D], BF16, tag="wv", name="wv")
        nc.sync.dma_start(out=wq_t, in_=wq_d.rearrange("(n p) d -> p n d", p=P))

        xh = x_pool.tile([P, NCH, T], BF16, tag="xh")
        for c in range(NCH):  # first t-quarter per c-chunk: starts matmuls early
            nc.sync.dma_start(
                out=xh[:, c, 0:512], in_=xT[c * P:(c + 1) * P, 0:512])
        nc.sync.dma_start(out=wk_t, in_=wk_d.rearrange("(n p) d -> p n d", p=P))
        for c in range(NCH):
            nc.sync.dma_start(
                out=xh[:, c, 512:1024], in_=xT[c * P:(c + 1) * P, 512:1024])
        nc.sync.dma_start(out=wv_t, in_=wv_d.rearrange("(n p) d -> p n d", p=P))
        nc.sync.dma_start(
            out=xh[:, :, 1024:2048],
            in_=xT[:, 1024:2048].rearrange("(n p) t -> p n t", p=P))

        wpt_t = [wpt_pool.tile([P, C], BF16, tag="wpt", name=f"wpt{i}")
                 for i in range(4)]
        for q in range(4):
            nc.sync.dma_start(out=wpt_t[q], in_=wpt_d[q * P:(q + 1) * P, :])
        bpc = bpc_pool.tile([P, NCH], F32)
        nc.sync.dma_start(out=bpc, in_=bp_d.rearrange("(n p) -> p n", p=P))

        # ---- persistent activation tiles ----
        qT = [q_pool.tile([P, T], BF16, tag="qT", name=f"qT{i}")
              for i in range(4)]
        kT = [k_pool.tile([P, T], BF16, tag="kT", name=f"kT{i}")
              for i in range(4)]
        # v: [s-part, s-block, head, d + ones]
        v_t = v_pool.tile([P, NSB, HL, D + 1], BF16, tag="v")
        nc.vector.memset(v_t[:, :, :, D:], 1.0)
        outcat = [oc_pool.tile([P, T], BF16, tag="outcat", name=f"outcat{i}")
                  for i in range(4)]

        def qk_half(pr, qq, which):
            """Project one 512-wide t-quarter of q OR k for pair pr."""
            wsl = slice(pr * P, (pr + 1) * P)
            t0 = qq * 512
            w_t, qkT = (wq_t, qT) if which == 0 else (wk_t, kT)
            ps = psQ.tile([P, 512], F32, tag="psQ", name="qkps")
            for c in range(NCH):
                nc.tensor.matmul(
                    ps, w_t[:, c, wsl], xh[:, c, t0:t0 + 512],
                    start=c == 0, stop=c == NCH - 1)
            nc.vector.tensor_copy(out=qkT[pr][:, t0:t0 + 512], in_=ps)

        def qk_q(pr, qq):
            qk_half(pr, qq, 0)
            qk_half(pr, qq, 1)

        def v_s(s):
            """Project v for s-block s (all 8 local heads)."""
            vps = psQ.tile([P, 512], F32, tag="psQ", name="vps")
            for c in range(NCH):
                nc.tensor.matmul(
                    vps, xh[:, c, s * P:(s + 1) * P], wv_t[:, c, :],
                    start=c == 0, stop=c == NCH - 1)
            nc.vector.tensor_copy(
                out=v_t[:, s:s + 1, :, 0:D],
                in_=vps.rearrange("p (o h d) -> p o h d", o=1, h=HL))

        def v_q(qq):
            for s in range(4 * qq, 4 * qq + 4):
                v_s(s)

        def attention(pair, tile_units=None):
            """tile_units: j -> list of ~1-2us PE work closures, dispensed
            evenly across the j-tile's s-blocks so the PE never idles long
            enough to drop its clock while ScalarE chews on exp."""
            zb = z_pool.tile([P, 3, 512], F32, tag="zb", name=f"zb{pair}")
            zbs[pair] = zb  # visible to this pair's own unit closures
            for j in range(NTT):
                units = tile_units(j) if tile_units else []
                nsb_j = 4 * (j + 1) if causal else NSB
                # dispense unit m before s-block floor(m*nsb/M)
                sched = {}
                for m, fn in enumerate(units):
                    sched.setdefault(m * nsb_j // max(len(units), 1),
                                     []).append(fn)
                outp = [psO.tile([D + 1, 512], F32, tag="psO",
                                 name=f"outp{u}") for u in range(2)]

                def emit_pv(i, lo, last):
                    for u in range(2):
                        nc.tensor.matmul(
                            outp[u][:, lo:512],
                            v_t[:, i, pair * 2 + u, :],
                            pend[i][:, u, lo:512],
                            start=(i == 0), stop=last,
                            skip_group_check=True)
                    del pend[i]

                pend = {}
                prev = None
                for i in range(nsb_j):
                    for fn in sched.get(i, ()):
                        fn()
                    r = i - 4 * j if causal else -1
                    lo = max(r, 0) * P
                    last = i == nsb_j - 1
                    scs = psA.tile([P, 2, 512], F32, tag="psA", name="scs")
                    pts = p_pool.tile([P, 2, 512], BF16, tag="pT", name="pts")
                    pend[i] = pts
                    for u in range(2):
                        dsl = slice(u * D, (u + 1) * D)
                        nc.tensor.matmul(
                            scs[:, u, lo:512],
                            kT[pair][dsl, i * P:(i + 1) * P],
                            qT[pair][dsl, j * 512 + lo:(j + 1) * 512],
                            start=True, stop=True)
                    if causal and r >= 0:
                        nc.vector.tensor_add(
                            scs[:, :, lo:lo + P], scs[:, :, lo:lo + P], mask)
                    nc.scalar.activation(
                        out=pts[:, :, lo:512], in_=scs[:, :, lo:512],
                        func=mybir.ActivationFunctionType.Exp, scale=SCALE)
                    if prev is not None:
                        emit_pv(*prev)
                    prev = (i, lo, last)
                if prev is not None:
                    emit_pv(*prev)
                for u in range(2):
                    # raw (unnormalized) head output + Z row gather
                    nc.vector.tensor_copy(
                        out=outcat[pair][u * D:(u + 1) * D,
                                         j * 512:(j + 1) * 512],
                        in_=outp[u][0:D, :])
                    k0, slot = _zslot(j, u)
                    nc.vector.tensor_copy(
                        out=zb[k0:k0 + 1, slot, :], in_=outp[u][D:D + 1, :])
            return zb

        rzbs = [None] * 4

        def recip_z(pair, zb, psl=slice(0, P), sl=slice(0, 3)):
            """Part A of normalization: 1/Z (DVE), f32->bf16.

            psl/sl select the zb region so pair 3 can normalize
            incrementally as its j-tiles finish without a dependency on
            later rows. Overlapping regions across calls rewrite the same
            values; the scheduler serializes them harmlessly.
            """
            if rzbs[pair] is None:
                rzbs[pair] = (
                    z_pool.tile([P, 3, 512], F32, tag="rz", name=f"rz{pair}"),
                    z_pool.tile([P, 3, 512], BF16, tag="rzb",
                                name=f"rzb{pair}"),
                )
            rz, rzb = rzbs[pair]
            nc.vector.reciprocal_approx_fast(
                out=rz[psl, sl, :], in_=zb[psl, sl, :])
            nc.vector.tensor_copy(out=rzb[psl, sl, :], in_=rz[psl, sl, :])

        def bcast_mul_1(pair, j, u):
            """Part B: broadcast 1/Z across partitions (K=1 matmul),
            scale outcat."""
            rzb = rzbs[pair][1]
            k0, slot = _zslot(j, u)
            bps = psQ.tile([P, 512], F32, tag="psQ", name="bps")
            nc.tensor.matmul(
                bps, ones_bc[k0:k0 + 1, :], rzb[k0:k0 + 1, slot, :],
                start=True, stop=True)
            osl = outcat[pair][u * D:(u + 1) * D, j * 512:(j + 1) * 512]
            nc.vector.tensor_mul(osl, osl, bps[u * D:(u + 1) * D, :])

        def bcast_mul(pair, js):
            for j in js:
                for u in range(2):
                    bcast_mul_1(pair, j, u)

        def yproj_ci(tc_, ci):
            """yT[c'-chunk, t-chunk] = sum_q wpt[q].T @ outcat[q]."""
            tg = slice(tc_ * 512, (tc_ + 1) * 512)
            yps = psQ.tile([P, 512], F32, tag="psQ", name="yps")
            for q in range(4):
                nc.tensor.matmul(
                    yps,
                    wpt_t[q][:, ci * P:(ci + 1) * P],
                    outcat[q][:, tg],
                    start=(q == 0), stop=(q == 3))
            yt = yst_pool.tile([P, 512], F32, tag="yst", name="yt")
            nc.scalar.activation(
                out=yt, in_=yps,
                func=mybir.ActivationFunctionType.Identity,
                bias=bpc[:, ci:ci + 1])
            nc.sync.dma_start(out=y_d[ci * P:(ci + 1) * P, tg], in_=yt)

        def yproj_chunk(tc_):
            for ci in range(NCH):
                yproj_ci(tc_, ci)

        def yproj_wide(tc0, ci):
            """Two t-chunks per PSUM tile: halves the identity/DMA count
            on the tail where nothing else is left to overlap."""
            yps = psA.tile([P, 2, 512], F32, tag="psA", name="ypsw")
            for w, tc_ in enumerate((tc0, tc0 + 1)):
                tg = slice(tc_ * 512, (tc_ + 1) * 512)
                for q in range(4):
                    nc.tensor.matmul(
                        yps[:, w, :],
                        wpt_t[q][:, ci * P:(ci + 1) * P],
                        outcat[q][:, tg],
                        start=(q == 0), stop=(q == 3))
            yt = yst_pool.tile([P, 2, 512], F32, tag="ystw", name="ytw")
            nc.scalar.activation(
                out=yt, in_=yps,
                func=mybir.ActivationFunctionType.Identity,
                bias=bpc[:, ci:ci + 1])
            nc.sync.dma_start(
                out=y_d[ci * P:(ci + 1) * P, tc0 * 512:(tc0 + 2) * 512],
                in_=yt.rearrange("p w t -> p (w t)"))

        # ---- schedule ----
        # The exp stream on ScalarE is the per-pair bottleneck; all other
        # PE work (same pair's next qk quarter, pair0's v s-blocks, the
        # previous pair's normalization, pair3's output projection) is
        # chopped into ~1-2us units dispensed evenly between s-blocks, so
        # the PE stays warm and ScalarE is never starved. v_s(s) units are
        # scheduled so slice s lands before the s-block that consumes it.
        zbs = [None] * 4
        if causal:
            qk_q(0, 0)

            def L(fn, *a):
                return lambda: fn(*a)

            def warm():
                # dependency-free PE activity: keeps the HAM clock-gate at
                # 2.4GHz through exp-bound stretches and dependency stalls
                # (~53ns; the next matmul self-loads its own weights anyway)
                nc.tensor.ldweights(weights=ones_bc)

            def make_units(pr):
                def units(j):
                    us = []
                    if pr == 0:
                        us += [L(v_s, s) for s in range(4 * j, 4 * j + 4)]
                    if j < 3:
                        us += [L(qk_half, pr, j + 1, 0),
                               L(qk_half, pr, j + 1, 1)]
                    elif pr < 3:
                        us += [L(qk_half, pr + 1, 0, 0),
                               L(qk_half, pr + 1, 0, 1)]
                    if pr > 0:
                        if j == 1:
                            us.append(L(recip_z, pr - 1, zbs[pr - 1]))
                        elif j == 2:
                            us += [L(bcast_mul_1, pr - 1, jj, u)
                                   for jj in range(NTT) for u in range(2)]
                    if pr == 3:
                        # incremental self-normalize + output projection
                        if j == 2:
                            us.append(L(recip_z, 3, zbs[3]))  # j0+j1 rows
                            us += [L(bcast_mul_1, 3, 0, u) for u in range(2)]
                        elif j == 3:
                            # j2 rows live at partition 32 only: the
                            # partition-sliced recip avoids waiting on j3
                            us.append(L(recip_z, 3, zbs[3], slice(32, 33),
                                        slice(1, 3)))
                            us += [L(bcast_mul_1, 3, jj, u)
                                   for jj in (1, 2) for u in range(2)]
                            us += [L(yproj_ci, 0, ci) for ci in range(NCH)]
                    return us
                return units

            for pr in range(4):
                attention(pr, tile_units=make_units(pr))
            recip_z(3, zbs[3], psl=slice(64, 65), sl=slice(0, 2))  # j3 rows
            for ci in range(NCH):
                yproj_wide(1, ci)  # t-chunks 1+2; recip3/j3 hides under it
            bcast_mul(3, (3,))
            yproj_chunk(3)
        else:
            # non-causal: every j reads all of kT/v, so project fully first
            for qq in range(4):
                qk_q(0, qq)
                v_q(qq)
            for pr in range(4):
                zbs[pr] = attention(pr)
                if pr < 3:
                    for qq in range(4):
                        qk_q(pr + 1, qq)
                if pr > 0:
                    recip_z(pr - 1, zbs[pr - 1])
                    bcast_mul(pr - 1, range(NTT))
            recip_z(3, zbs[3])
            bcast_mul(3, range(NTT))
            for tc_ in range(NTT):
                yproj_chunk(tc_)


_NC_CACHE = {}
LAST_RESULTS = None


def kernel(x, Wq, Wk, Wv, Wp, bp, is_masked, **_unused):
    global LAST_RESULTS
    from ml_dtypes import bfloat16

    x = np.asarray(x, np.float32)
    Wq = np.asarray(Wq, np.float32)
    Wk = np.asarray(Wk, np.float32)
    Wv = np.asarray(Wv, np.float32)
    Wp = np.asarray(Wp, np.float32)
    bp = np.asarray(bp, np.float32)
    causal = bool(np.asarray(is_masked).item())

    if causal not in _NC_CACHE:
        _NC_CACHE[causal] = _build(causal)
    nc = _NC_CACHE[causal]

    # host-side layout prep
    wq_r = np.ascontiguousarray(Wq.transpose(1, 0, 2).reshape(C, H * D))
    wk_r = np.ascontiguousarray(Wk.transpose(1, 0, 2).reshape(C, H * D))
    wv_r = np.ascontiguousarray(Wv.transpose(1, 0, 2).reshape(C, H * D))
    wpt = np.ascontiguousarray(Wp.T)
    zeros = np.zeros_like(bp)

    xTs = [np.ascontiguousarray(x[b].T).astype(bfloat16) for b in range(B)]
    in_maps = []
    for core in range(8):
        b, hh = core // 2, core % 2
        csl = slice(hh * HL * D, (hh + 1) * HL * D)
        in_maps.append({
            "xT": xTs[b],
            "wq": np.ascontiguousarray(wq_r[:, csl]).astype(bfloat16),
            "wk": np.ascontiguousarray(wk_r[:, csl]).astype(bfloat16),
            "wv": np.ascontiguousarray(wv_r[:, csl]).astype(bfloat16),
            "wpt": np.ascontiguousarray(wpt[csl, :]).astype(bfloat16),
            "bp": bp if hh == 0 else zeros,
        })

    trace = bool(int(os.environ.get("KERNEL_TRACE", "0")))
    res = run_bass_kernel_spmd(
        nc, in_maps, core_ids=list(range(8)), trace=trace)
    LAST_RESULTS = res

    y = np.empty((B, T, C), np.float32)
    for b in range(B):
        y[b] = res.results[2 * b]["y"].T + res.results[2 * b + 1]["y"].T
    return y


# revision 38
# speedup vs baseline: 1.1605x; 1.1605x over previous
"""Multi-head attention (B=4, T=2048, C=1024, H=16, D=64) on 8 TRN2 cores.

Sharding: core i handles batch b=i//2 and the 8 heads of half hh=i%2.
Each core computes its heads' contribution through the row-sharded output
projection -> partial yT [C, T]; host transposes and sums the two partials
per batch.

v3: all matmul operands bf16 (fp32 "HIGH-mode" matmuls run at ~half PE rate
and block FastWeightLoad). The softmax exp on ScalarE (~157us) is the
per-pair bottleneck, so all PE work that is not on the exp critical path
(q/k projection quarters of the SAME pair one t-tile ahead, v projection
s-quarters, the deferred normalization of the PREVIOUS pair) is interleaved
into the attention j-loop where the PE otherwise idles waiting for exp.
Output projection emits yT = wpt_chunk.T @ outcat so the bias add becomes a
per-partition scalar on the (idle by then) ScalarE.

Per-core layouts (host pre-arranged, bf16):
  xT  [C, T]    = x[b].T
  wq/wk/wv [C, 512]  columns = (local head)*64 + d
  wpt [512, C]  rows  = (local head)*64 + d   (= Wp.T row-slice)
  bp  [C] f32   bias on even cores, zeros on odd (summed partials)
Output: yT [C, T] f32 (host transposes).

On-chip dataflow per core:
  qT/kT [128, T] per head-pair via lhsT=w-chunk, rhs=xT-chunk (N=512)
  v     [s, h, d] natural via lhsT=xT s-slice, rhs=wv (N=512, all 8 heads)
  scoresT[s,t]: lhsT=kT s-block [64,128], rhs=qT t-tile [64,<=512],
                2 heads row-tiled (K=64 each, concurrent on the PE array)
  exp on ScalarE PSUM->SBUF bf16 with scale=1/sqrt(C); causal via additive
  -1e9 mask on the straddling 128-blocks
  PV: lhsT=[v ; ones] [128,65] bf16, rhs=pT -> outT [65,512] PSUM per head,
  accumulated over s-blocks; row 64 = softmax normalizer Z
  normalize: reciprocal_approx_fast(Z) -> bf16 -> per-row K=1 matmul
  broadcast -> DVE mult into outcat (deferred one pair)
  yT: lhsT=wpt c'-chunk, rhs=outcat t-chunk; bias via ScalarE Identity
"""

import os
import sys

import numpy as np

for _p in ("/opt/trn_rl_repo", "/root/.axon_site/_ro/trn_rl_repo"):
    if os.path.isdir(_p) and _p not in sys.path:
        sys.path.append(_p)

import concourse.bass as bass
import concourse.bacc as bacc
import concourse.mybir as mybir
import concourse.tile as tile
from concourse.bass_utils import run_bass_kernel_spmd

B, T, C, H, D = 4, 2048, 1024, 16, 64
HL = H // 2          # heads per core
P = 128
NCH = C // P         # 8 c-chunks
NTT = T // 512       # 4 t-tiles of 512
NSB = T // P         # 16 s-blocks of 128
SCALE = 1.0 / 32.0   # 1/sqrt(C)

F32 = mybir.dt.float32
BF16 = mybir.dt.bfloat16

# zb row (j,u) -> idx=2j+u at partition 32*(idx//3), slot idx%3
# (AP base partitions are restricted to {0,32,64})
def _zslot(j, u):
    idx = 2 * j + u
    return 32 * (idx // 3), idx % 3


def _build(causal: bool) -> bass.Bass:
    nc = bacc.Bacc("TRN2", target_bir_lowering=False, debug=False, num_devices=8)

    xT = nc.dram_tensor("xT", [C, T], BF16, kind="ExternalInput").ap()
    wq_d = nc.dram_tensor("wq", [C, HL * D], BF16, kind="ExternalInput").ap()
    wk_d = nc.dram_tensor("wk", [C, HL * D], BF16, kind="ExternalInput").ap()
    wv_d = nc.dram_tensor("wv", [C, HL * D], BF16, kind="ExternalInput").ap()
    wpt_d = nc.dram_tensor("wpt", [HL * D, C], BF16, kind="ExternalInput").ap()
    bp_d = nc.dram_tensor("bp", [C], F32, kind="ExternalInput").ap()
    y_d = nc.dram_tensor("y", [C, T], F32, kind="ExternalOutput").ap()

    with tile.TileContext(nc) as tc:
        _emit(nc, tc, causal, xT, wq_d, wk_d, wv_d, wpt_d, bp_d, y_d)
    nc.compile()
    return nc


def _emit(nc, tc, causal, xT, wq_d, wk_d, wv_d, wpt_d, bp_d, y_d):
    from contextlib import ExitStack

    ctx = ExitStack()
    with ctx:
        consts = ctx.enter_context(tc.tile_pool(name="consts", bufs=1))
        x_pool = ctx.enter_context(tc.tile_pool(name="xh", bufs=1))
        w_pool = ctx.enter_context(tc.tile_pool(name="w", bufs=1))
        q_pool = ctx.enter_context(tc.tile_pool(name="qT", bufs=3))
        k_pool = ctx.enter_context(tc.tile_pool(name="kT", bufs=3))
        v_pool = ctx.enter_context(tc.tile_pool(name="v", bufs=1))
        oc_pool = ctx.enter_context(tc.tile_pool(name="outcat", bufs=4))
        p_pool = ctx.enter_context(tc.tile_pool(name="pT", bufs=4))
        z_pool = ctx.enter_context(tc.tile_pool(name="zb", bufs=2))
        wpt_pool = ctx.enter_context(tc.tile_pool(name="wpt", bufs=4))
        bpc_pool = ctx.enter_context(tc.tile_pool(name="bpc", bufs=1))
        yst_pool = ctx.enter_context(tc.tile_pool(name="yst", bufs=3))
        bps_pool = ctx.enter_context(tc.tile_pool(name="bps", bufs=3))
        psA = ctx.enter_context(tc.tile_pool(name="psA", bufs=2, space="PSUM"))
        psO = ctx.enter_context(tc.tile_pool(name="psO", bufs=2, space="PSUM"))
        psQ = ctx.enter_context(tc.tile_pool(name="psQ", bufs=2, space="PSUM"))

        # ---- constants ----
        # additive causal mask: 0 where free>=partition else -1e9
        mask = None
        if causal:
            mask = consts.tile([P, 2, P], F32)
            nc.vector.memset(mask, 0.0)
            for _u in range(2):
                nc.gpsimd.affine_select(
                    out=mask[:, _u, :], in_=mask[:, _u, :],
                    compare_op=mybir.AluOpType.is_ge,
                    fill=-1e9, base=0,
                    pattern=[[1, P]], channel_multiplier=-1,
                )
        ones_bc = consts.tile([P, P], BF16)
        nc.vector.memset(ones_bc, 1.0)

        # ---- DMA: weights first (first matmuls need them), x in c-chunks ----
        wq_t = w_pool.tile([P, NCH, HL * D], BF16, tag="wq", name="wq")
        wk_t = w_pool.tile([P, NCH, HL * D], BF16, tag="wk", name="wk")
        wv_t = w_pool.tile([P, NCH, HL * D], BF16, tag="wv", name="wv")
        nc.sync.dma_start(out=wq_t, in_=wq_d.rearrange("(n p) d -> p n d", p=P))

        xh = x_pool.tile([P, NCH, T], BF16, tag="xh")
        for c in range(NCH):  # first t-quarter per c-chunk: starts matmuls early
            nc.sync.dma_start(
                out=xh[:, c, 0:512], in_=xT[c * P:(c + 1) * P, 0:512])
        nc.sync.dma_start(out=wk_t, in_=wk_d.rearrange("(n p) d -> p n d", p=P))
        for c in range(NCH):
            nc.sync.dma_start(
                out=xh[:, c, 512:1024], in_=xT[c * P:(c + 1) * P, 512:1024])
        nc.sync.dma_start(out=wv_t, in_=wv_d.rearrange("(n p) d -> p n d", p=P))
        nc.sync.dma_start(
            out=xh[:, :, 1024:2048],
            in_=xT[:, 1024:2048].rearrange("(n p) t -> p n t", p=P))

        wpt_t = [wpt_pool.tile([P, C], BF16, tag="wpt", name=f"wpt{i}")
                 for i in range(4)]
        for q in range(4):
            nc.sync.dma_start(out=wpt_t[q], in_=wpt_d[q * P:(q + 1) * P, :])
        bpc = bpc_pool.tile([P, NCH], F32)
        nc.sync.dma_start(out=bpc, in_=bp_d.rearrange("(n p) -> p n", p=P))

        # ---- persistent activation tiles ----
        qT = [q_pool.tile([P, T], BF16, tag="qT", name=f"qT{i}")
              for i in range(4)]
        kT = [k_pool.tile([P, T], BF16, tag="kT", name=f"kT{i}")
              for i in range(4)]
        # v: [s-part, s-block, head, d + ones]
        v_t = v_pool.tile([P, NSB, HL, D + 1], BF16, tag="v")
        nc.vector.memset(v_t[:, :, :, D:], 1.0)
        outcat = [oc_pool.tile([P, T], BF16, tag="outcat", name=f"outcat{i}")
                  for i in range(4)]

        def qk_half(pr, qq, which):
            """Project one 512-wide t-quarter of q OR k for pair pr."""
            wsl = slice(pr * P, (pr + 1) * P)
            t0 = qq * 512
            w_t, qkT = (wq_t, qT) if which == 0 else (wk_t, kT)
            ps = psQ.tile([P, 512], F32, tag="psQ", name="qkps")
            for c in range(NCH):
                nc.tensor.matmul(
                    ps, w_t[:, c, wsl], xh[:, c, t0:t0 + 512],
                    start=c == 0, stop=c == NCH - 1)
            nc.vector.tensor_copy(out=qkT[pr][:, t0:t0 + 512], in_=ps)

        def qk_q(pr, qq):
            qk_half(pr, qq, 0)
            qk_half(pr, qq, 1)

        def v_s(s):
            """Project v for s-block s (all 8 local heads)."""
            vps = psQ.tile([P, 512], F32, tag="psQ", name="vps")
            for c in range(NCH):
                nc.tensor.matmul(
                    vps, xh[:, c, s * P:(s + 1) * P], wv_t[:, c, :],
                    start=c == 0, stop=c == NCH - 1)
            nc.vector.tensor_copy(
                out=v_t[:, s:s + 1, :, 0:D],
                in_=vps.rearrange("p (o h d) -> p o h d", o=1, h=HL))

        def v_q(qq):
            for s in range(4 * qq, 4 * qq + 4):
                v_s(s)

        def attention(pair, tile_units=None):
            """tile_units: j -> list of ~1-2us PE work closures, dispensed
            evenly across the j-tile's s-blocks so the PE never idles long
            enough to drop its clock while ScalarE chews on exp."""
            zb = z_pool.tile([P, 3, 512], F32, tag="zb", name=f"zb{pair}")
            zbs[pair] = zb  # visible to this pair's own unit closures
            for j in range(NTT):
                units = tile_units(j) if tile_units else []
                nsb_j = 4 * (j + 1) if causal else NSB
                # dispense unit m before s-block floor(m*nsb/M)
                sched = {}
                for m, fn in enumerate(units):
                    sched.setdefault(m * nsb_j // max(len(units), 1),
                                     []).append(fn)
                outp = [psO.tile([D + 1, 512], F32, tag="psO",
                                 name=f"outp{u}") for u in range(2)]

                def emit_pv(i, lo, last):
                    for u in range(2):
                        nc.tensor.matmul(
                            outp[u][:, lo:512],
                            v_t[:, i, pair * 2 + u, :],
                            pend[i][:, u, lo:512],
                            start=(i == 0), stop=last,
                            skip_group_check=True)
                    del pend[i]

                pend = {}
                prev = None
                for i in range(nsb_j):
                    for fn in sched.get(i, ()):
                        fn()
                    r = i - 4 * j if causal else -1
                    lo = max(r, 0) * P
                    last = i == nsb_j - 1
                    scs = psA.tile([P, 2, 512], F32, tag="psA", name="scs")
                    pts = p_pool.tile([P, 2, 512], BF16, tag="pT", name="pts")
                    pend[i] = pts
                    for u in range(2):
                        dsl = slice(u * D, (u + 1) * D)
                        nc.tensor.matmul(
                            scs[:, u, lo:512],
                            kT[pair][dsl, i * P:(i + 1) * P],
                            qT[pair][dsl, j * 512 + lo:(j + 1) * 512],
                            start=True, stop=True)
                    if causal and r >= 0:
                        nc.vector.tensor_add(
                            scs[:, :, lo:lo + P], scs[:, :, lo:lo + P], mask)
                    nc.scalar.activation(
                        out=pts[:, :, lo:512], in_=scs[:, :, lo:512],
                        func=mybir.ActivationFunctionType.Exp, scale=SCALE)
                    if prev is not None:
                        emit_pv(*prev)
                    prev = (i, lo, last)
                if prev is not None:
                    emit_pv(*prev)
                for u in range(2):
                    # raw (unnormalized) head output + Z row gather
                    nc.vector.tensor_copy(
                        out=outcat[pair][u * D:(u + 1) * D,
                                         j * 512:(j + 1) * 512],
                        in_=outp[u][0:D, :])
                    k0, slot = _zslot(j, u)
                    nc.vector.tensor_copy(
                        out=zb[k0:k0 + 1, slot, :], in_=outp[u][D:D + 1, :])
            return zb

        rzbs = [None] * 4

        def recip_z(pair, zb, psl=slice(0, P), sl=slice(0, 3)):
            """Part A of normalization: 1/Z (DVE), f32->bf16.

            psl/sl select the zb region so pair 3 can normalize
            incrementally as its j-tiles finish without a dependency on
            later rows. Overlapping regions across calls rewrite the same
            values; the scheduler serializes them harmlessly.
            """
            if rzbs[pair] is None:
                rzbs[pair] = (
                    z_pool.tile([P, 3, 512], F32, tag="rz", name=f"rz{pair}"),
                    z_pool.tile([P, 3, 512], BF16, tag="rzb",
                                name=f"rzb{pair}"),
                )
            rz, rzb = rzbs[pair]
            nc.vector.reciprocal_approx_fast(
                out=rz[psl, sl, :], in_=zb[psl, sl, :])
            nc.vector.tensor_copy(out=rzb[psl, sl, :], in_=rz[psl, sl, :])

        def bcast_mul_1(pair, j, u):
            """Part B: broadcast 1/Z across partitions (K=1 matmul),
            scale outcat."""
            rzb = rzbs[pair][1]
            k0, slot = _zslot(j, u)
            bps = psQ.tile([P, 512], F32, tag="psQ", name="bps")
            nc.tensor.matmul(
                bps, ones_bc[k0:k0 + 1, :], rzb[k0:k0 + 1, slot, :],
                start=True, stop=True)
            osl = outcat[pair][u * D:(u + 1) * D, j * 512:(j + 1) * 512]
            nc.vector.tensor_mul(osl, osl, bps[u * D:(u + 1) * D, :])

        def bcast_mul(pair, js):
            for j in js:
                for u in range(2):
                    bcast_mul_1(pair, j, u)

        def yproj_ci(tc_, ci):
            """yT[c'-chunk, t-chunk] = sum_q wpt[q].T @ outcat[q]."""
            tg = slice(tc_ * 512, (tc_ + 1) * 512)
            yps = psQ.tile([P, 512], F32, tag="psQ", name="yps")
            for q in range(4):
                nc.tensor.matmul(
                    yps,
                    wpt_t[q][:, ci * P:(ci + 1) * P],
                    outcat[q][:, tg],
                    start=(q == 0), stop=(q == 3))
            yt = yst_pool.tile([P, 512], F32, tag="yst", name="yt")
            nc.scalar.activation(
                out=yt, in_=yps,
                func=mybir.ActivationFunctionType.Identity,
                bias=bpc[:, ci:ci + 1])
            nc.sync.dma_start(out=y_d[ci * P:(ci + 1) * P, tg], in_=yt)

        def yproj_chunk(tc_):
            for ci in range(NCH):
                yproj_ci(tc_, ci)

        def yproj_wide(tc0, ci):
            """Two t-chunks per PSUM tile: halves the identity/DMA count
            on the tail where nothing else is left to overlap."""
            yps = psA.tile([P, 2, 512], F32, tag="psA", name="ypsw")
            for w, tc_ in enumerate((tc0, tc0 + 1)):
                tg = slice(tc_ * 512, (tc_ + 1) * 512)
                for q in range(4):
                    nc.tensor.matmul(
                        yps[:, w, :],
                        wpt_t[q][:, ci * P:(ci + 1) * P],
                        outcat[q][:, tg],
                        start=(q == 0), stop=(q == 3))
            yt = yst_pool.tile([P, 2, 512], F32, tag="ystw", name="ytw")
            nc.scalar.activation(
                out=yt, in_=yps,
                func=mybir.ActivationFunctionType.Identity,
                bias=bpc[:, ci:ci + 1])
            nc.sync.dma_start(
                out=y_d[ci * P:(ci + 1) * P, tc0 * 512:(tc0 + 2) * 512],
                in_=yt.rearrange("p w t -> p (w t)"))

        # ---- schedule ----
        # The exp stream on ScalarE is the per-pair bottleneck; all other
        # PE work (same pair's next qk quarter, pair0's v s-blocks, the
        # previous pair's normalization, pair3's output projection) is
        # chopped into ~1-2us units dispensed evenly between s-blocks, so
        # the PE stays warm and ScalarE is never starved. v_s(s) units are
        # scheduled so slice s lands before the s-block that consumes it.
        zbs = [None] * 4
        if causal:
            qk_q(0, 0)

            def L(fn, *a):
                return lambda: fn(*a)

            def warm():
                # dependency-free PE activity: keeps the HAM clock-gate at
                # 2.4GHz through exp-bound stretches and dependency stalls
                # (~53ns; the next matmul self-loads its own weights anyway)
                nc.tensor.ldweights(weights=ones_bc)

            def make_units(pr):
                def units(j):
                    us = []
                    if pr == 0:
                        us += [L(v_s, s) for s in range(4 * j, 4 * j + 4)]
                    if j < 3:
                        us += [L(qk_half, pr, j + 1, 0),
                               L(qk_half, pr, j + 1, 1)]
                    elif pr < 3:
                        us += [L(qk_half, pr + 1, 0, 0),
                               L(qk_half, pr + 1, 0, 1)]
                    if pr > 0:
                        if j == 1:
                            us.append(L(recip_z, pr - 1, zbs[pr - 1]))
                        elif j == 2:
                            us += [L(bcast_mul_1, pr - 1, jj, u)
                                   for jj in range(NTT) for u in range(2)]
                    if pr == 3:
                        # incremental self-normalize + output projection
                        if j == 2:
                            us.append(L(recip_z, 3, zbs[3]))  # j0+j1 rows
                            us += [L(bcast_mul_1, 3, 0, u) for u in range(2)]
                        elif j == 3:
                            # j2 rows live at partition 32 only: the
                            # partition-sliced recip avoids waiting on j3
                            us.append(L(recip_z, 3, zbs[3], slice(32, 33),
                                        slice(1, 3)))
                            us += [L(bcast_mul_1, 3, jj, u)
                                   for jj in (1, 2) for u in range(2)]
                            us += [L(yproj_ci, 0, ci) for ci in range(NCH)]
                    return us
                return units

            for pr in range(4):
                attention(pr, tile_units=make_units(pr))
            recip_z(3, zbs[3], psl=slice(64, 65), sl=slice(0, 2))  # j3 rows
            for ci in range(4):
                yproj_wide(1, ci)  # t-chunks 1+2; recip3/j3 hides under it
            bcast_mul(3, (3,))  # j3 muls run while the PE continues below
            for ci in range(4, NCH):
                yproj_wide(1, ci)
            yproj_chunk(3)
        else:
            # non-causal: every j reads all of kT/v, so project fully first
            for qq in range(4):
                qk_q(0, qq)
                v_q(qq)
            for pr in range(4):
                zbs[pr] = attention(pr)
                if pr < 3:
                    for qq in range(4):
                        qk_q(pr + 1, qq)
                if pr > 0:
                    recip_z(pr - 1, zbs[pr - 1])
                    bcast_mul(pr - 1, range(NTT))
            recip_z(3, zbs[3])
            bcast_mul(3, range(NTT))
            for tc_ in range(NTT):
                yproj_chunk(tc_)


_NC_CACHE = {}
LAST_RESULTS = None


def kernel(x, Wq, Wk, Wv, Wp, bp, is_masked, **_unused):
    global LAST_RESULTS
    from ml_dtypes import bfloat16

    x = np.asarray(x, np.float32)
    Wq = np.asarray(Wq, np.float32)
    Wk = np.asarray(Wk, np.float32)
    Wv = np.asarray(Wv, np.float32)
    Wp = np.asarray(Wp, np.float32)
    bp = np.asarray(bp, np.float32)
    causal = bool(np.asarray(is_masked).item())

    if causal not in _NC_CACHE:
        _NC_CACHE[causal] = _build(causal)
    nc = _NC_CACHE[causal]

    # host-side layout prep
    wq_r = np.ascontiguousarray(Wq.transpose(1, 0, 2).reshape(C, H * D))
    wk_r = np.ascontiguousarray(Wk.transpose(1, 0, 2).reshape(C, H * D))
    wv_r = np.ascontiguousarray(Wv.transpose(1, 0, 2).reshape(C, H * D))
    wpt = np.ascontiguousarray(Wp.T)
    zeros = np.zeros_like(bp)

    xTs = [np.ascontiguousarray(x[b].T).astype(bfloat16) for b in range(B)]
    in_maps = []
    for core in range(8):
        b, hh = core // 2, core % 2
        csl = slice(hh * HL * D, (hh + 1) * HL * D)
        in_maps.append({
            "xT": xTs[b],
            "wq": np.ascontiguousarray(wq_r[:, csl]).astype(bfloat16),
            "wk": np.ascontiguousarray(wk_r[:, csl]).astype(bfloat16),
            "wv": np.ascontiguousarray(wv_r[:, csl]).astype(bfloat16),
            "wpt": np.ascontiguousarray(wpt[csl, :]).astype(bfloat16),
            "bp": bp if hh == 0 else zeros,
        })

    trace = bool(int(os.environ.get("KERNEL_TRACE", "0")))
    res = run_bass_kernel_spmd(
        nc, in_maps, core_ids=list(range(8)), trace=trace)
    LAST_RESULTS = res

    y = np.empty((B, T, C), np.float32)
    for b in range(B):
        y[b] = res.results[2 * b]["y"].T + res.results[2 * b + 1]["y"].T
    return y


# revision 39
# speedup vs baseline: 1.1673x; 1.0059x over previous
"""Multi-head attention (B=4, T=2048, C=1024, H=16, D=64) on 8 TRN2 cores.

Sharding: core i handles batch b=i//2 and the 8 heads of half hh=i%2.
Each core computes its heads' contribution through the row-sharded output
projection -> partial yT [C, T]; host transposes and sums the two partials
per batch.

v3: all matmul operands bf16 (fp32 "HIGH-mode" matmuls run at ~half PE rate
and block FastWeightLoad). The softmax exp on ScalarE (~157us) is the
per-pair bottleneck, so all PE work that is not on the exp critical path
(q/k projection quarters of the SAME pair one t-tile ahead, v projection
s-quarters, the deferred normalization of the PREVIOUS pair) is interleaved
into the attention j-loop where the PE otherwise idles waiting for exp.
Output projection emits yT = wpt_chunk.T @ outcat so the bias add becomes a
per-partition scalar on the (idle by then) ScalarE.

Per-core layouts (host pre-arranged, bf16):
  xT  [C, T]    = x[b].T
  wq/wk/wv [C, 512]  columns = (local head)*64 + d
  wpt [512, C]  rows  = (local head)*64 + d   (= Wp.T row-slice)
  bp  [C] f32   bias on even cores, zeros on odd (summed partials)
Output: yT [C, T] f32 (host transposes).

On-chip dataflow per core:
  qT/kT [128, T] per head-pair via lhsT=w-chunk, rhs=xT-chunk (N=512)
  v     [s, h, d] natural via lhsT=xT s-slice, rhs=wv (N=512, all 8 heads)
  scoresT[s,t]: lhsT=kT s-block [64,128], rhs=qT t-tile [64,<=512],
                2 heads row-tiled (K=64 each, concurrent on the PE array)
  exp on ScalarE PSUM->SBUF bf16 with scale=1/sqrt(C); causal via additive
  -1e9 mask on the straddling 128-blocks
  PV: lhsT=[v ; ones] [128,65] bf16, rhs=pT -> outT [65,512] PSUM per head,
  accumulated over s-blocks; row 64 = softmax normalizer Z
  normalize: reciprocal_approx_fast(Z) -> bf16 -> per-row K=1 matmul
  broadcast -> DVE mult into outcat (deferred one pair)
  yT: lhsT=wpt c'-chunk, rhs=outcat t-chunk; bias via ScalarE Identity
"""

import os
import sys

import numpy as np

for _p in ("/opt/trn_rl_repo", "/root/.axon_site/_ro/trn_rl_repo"):
    if os.path.isdir(_p) and _p not in sys.path:
        sys.path.append(_p)

import concourse.bass as bass
import concourse.bacc as bacc
import concourse.mybir as mybir
import concourse.tile as tile
from concourse.bass_utils import run_bass_kernel_spmd

B, T, C, H, D = 4, 2048, 1024, 16, 64
HL = H // 2          # heads per core
P = 128
NCH = C // P         # 8 c-chunks
NTT = T // 512       # 4 t-tiles of 512
NSB = T // P         # 16 s-blocks of 128
SCALE = 1.0 / 32.0   # 1/sqrt(C)

F32 = mybir.dt.float32
BF16 = mybir.dt.bfloat16

# zb row (j,u) -> idx=2j+u at partition 32*(idx//3), slot idx%3
# (AP base partitions are restricted to {0,32,64})
def _zslot(j, u):
    idx = 2 * j + u
    return 32 * (idx // 3), idx % 3


def _build(causal: bool) -> bass.Bass:
    nc = bacc.Bacc("TRN2", target_bir_lowering=False, debug=False, num_devices=8)

    xT = nc.dram_tensor("xT", [C, T], BF16, kind="ExternalInput").ap()
    wq_d = nc.dram_tensor("wq", [C, HL * D], BF16, kind="ExternalInput").ap()
    wk_d = nc.dram_tensor("wk", [C, HL * D], BF16, kind="ExternalInput").ap()
    wv_d = nc.dram_tensor("wv", [C, HL * D], BF16, kind="ExternalInput").ap()
    wpt_d = nc.dram_tensor("wpt", [HL * D, C], BF16, kind="ExternalInput").ap()
    bp_d = nc.dram_tensor("bp", [C], F32, kind="ExternalInput").ap()
    y_d = nc.dram_tensor("y", [C, T], F32, kind="ExternalOutput").ap()

    with tile.TileContext(nc) as tc:
        _emit(nc, tc, causal, xT, wq_d, wk_d, wv_d, wpt_d, bp_d, y_d)
    nc.compile()
    return nc


def _emit(nc, tc, causal, xT, wq_d, wk_d, wv_d, wpt_d, bp_d, y_d):
    from contextlib import ExitStack

    ctx = ExitStack()
    with ctx:
        consts = ctx.enter_context(tc.tile_pool(name="consts", bufs=1))
        x_pool = ctx.enter_context(tc.tile_pool(name="xh", bufs=1))
        w_pool = ctx.enter_context(tc.tile_pool(name="w", bufs=1))
        q_pool = ctx.enter_context(tc.tile_pool(name="qT", bufs=3))
        k_pool = ctx.enter_context(tc.tile_pool(name="kT", bufs=3))
        v_pool = ctx.enter_context(tc.tile_pool(name="v", bufs=1))
        oc_pool = ctx.enter_context(tc.tile_pool(name="outcat", bufs=4))
        p_pool = ctx.enter_context(tc.tile_pool(name="pT", bufs=4))
        z_pool = ctx.enter_context(tc.tile_pool(name="zb", bufs=2))
        wpt_pool = ctx.enter_context(tc.tile_pool(name="wpt", bufs=4))
        bpc_pool = ctx.enter_context(tc.tile_pool(name="bpc", bufs=1))
        yst_pool = ctx.enter_context(tc.tile_pool(name="yst", bufs=3))
        bps_pool = ctx.enter_context(tc.tile_pool(name="bps", bufs=3))
        psA = ctx.enter_context(tc.tile_pool(name="psA", bufs=2, space="PSUM"))
        psO = ctx.enter_context(tc.tile_pool(name="psO", bufs=2, space="PSUM"))
        psQ = ctx.enter_context(tc.tile_pool(name="psQ", bufs=2, space="PSUM"))

        # ---- constants ----
        # additive causal mask: 0 where free>=partition else -1e9
        mask = None
        if causal:
            mask = consts.tile([P, 2, P], F32)
            nc.vector.memset(mask, 0.0)
            for _u in range(2):
                nc.gpsimd.affine_select(
                    out=mask[:, _u, :], in_=mask[:, _u, :],
                    compare_op=mybir.AluOpType.is_ge,
                    fill=-1e9, base=0,
                    pattern=[[1, P]], channel_multiplier=-1,
                )
        ones_bc = consts.tile([P, P], BF16)
        nc.vector.memset(ones_bc, 1.0)

        # ---- DMA: weights first (first matmuls need them), x in c-chunks ----
        wq_t = w_pool.tile([P, NCH, HL * D], BF16, tag="wq", name="wq")
        wk_t = w_pool.tile([P, NCH, HL * D], BF16, tag="wk", name="wk")
        wv_t = w_pool.tile([P, NCH, HL * D], BF16, tag="wv", name="wv")
        nc.sync.dma_start(out=wq_t, in_=wq_d.rearrange("(n p) d -> p n d", p=P))

        xh = x_pool.tile([P, NCH, T], BF16, tag="xh")
        for c in range(NCH):  # first t-quarter per c-chunk: starts matmuls early
            nc.sync.dma_start(
                out=xh[:, c, 0:512], in_=xT[c * P:(c + 1) * P, 0:512])
        nc.sync.dma_start(out=wk_t, in_=wk_d.rearrange("(n p) d -> p n d", p=P))
        for c in range(NCH):
            nc.sync.dma_start(
                out=xh[:, c, 512:1024], in_=xT[c * P:(c + 1) * P, 512:1024])
        nc.sync.dma_start(out=wv_t, in_=wv_d.rearrange("(n p) d -> p n d", p=P))
        nc.sync.dma_start(
            out=xh[:, :, 1024:2048],
            in_=xT[:, 1024:2048].rearrange("(n p) t -> p n t", p=P))

        wpt_t = [wpt_pool.tile([P, C], BF16, tag="wpt", name=f"wpt{i}")
                 for i in range(4)]
        for q in range(4):
            nc.sync.dma_start(out=wpt_t[q], in_=wpt_d[q * P:(q + 1) * P, :])
        bpc = bpc_pool.tile([P, NCH], F32)
        nc.sync.dma_start(out=bpc, in_=bp_d.rearrange("(n p) -> p n", p=P))

        # ---- persistent activation tiles ----
        qT = [q_pool.tile([P, T], BF16, tag="qT", name=f"qT{i}")
              for i in range(4)]
        kT = [k_pool.tile([P, T], BF16, tag="kT", name=f"kT{i}")
              for i in range(4)]
        # v: [s-part, s-block, head, d + ones]
        v_t = v_pool.tile([P, NSB, HL, D + 1], BF16, tag="v")
        nc.vector.memset(v_t[:, :, :, D:], 1.0)
        outcat = [oc_pool.tile([P, T], BF16, tag="outcat", name=f"outcat{i}")
                  for i in range(4)]

        def qk_half(pr, qq, which):
            """Project one 512-wide t-quarter of q OR k for pair pr."""
            wsl = slice(pr * P, (pr + 1) * P)
            t0 = qq * 512
            w_t, qkT = (wq_t, qT) if which == 0 else (wk_t, kT)
            ps = psQ.tile([P, 512], F32, tag="psQ", name="qkps")
            for c in range(NCH):
                nc.tensor.matmul(
                    ps, w_t[:, c, wsl], xh[:, c, t0:t0 + 512],
                    start=c == 0, stop=c == NCH - 1)
            nc.vector.tensor_copy(out=qkT[pr][:, t0:t0 + 512], in_=ps)

        def qk_q(pr, qq):
            qk_half(pr, qq, 0)
            qk_half(pr, qq, 1)

        def v_s(s):
            """Project v for s-block s (all 8 local heads)."""
            vps = psQ.tile([P, 512], F32, tag="psQ", name="vps")
            for c in range(NCH):
                nc.tensor.matmul(
                    vps, xh[:, c, s * P:(s + 1) * P], wv_t[:, c, :],
                    start=c == 0, stop=c == NCH - 1)
            nc.vector.tensor_copy(
                out=v_t[:, s:s + 1, :, 0:D],
                in_=vps.rearrange("p (o h d) -> p o h d", o=1, h=HL))

        def v_q(qq):
            for s in range(4 * qq, 4 * qq + 4):
                v_s(s)

        def attention(pair, tile_units=None):
            """tile_units: j -> list of ~1-2us PE work closures, dispensed
            evenly across the j-tile's s-blocks so the PE never idles long
            enough to drop its clock while ScalarE chews on exp."""
            zb = z_pool.tile([P, 3, 512], F32, tag="zb", name=f"zb{pair}")
            zbs[pair] = zb  # visible to this pair's own unit closures
            for j in range(NTT):
                units = tile_units(j) if tile_units else []
                nsb_j = 4 * (j + 1) if causal else NSB
                # dispense unit m before s-block floor(m*nsb/M)
                sched = {}
                for m, fn in enumerate(units):
                    sched.setdefault(m * nsb_j // max(len(units), 1),
                                     []).append(fn)
                outp = [psO.tile([D + 1, 512], F32, tag="psO",
                                 name=f"outp{u}") for u in range(2)]

                def emit_pv(i, lo, last):
                    for u in range(2):
                        nc.tensor.matmul(
                            outp[u][:, lo:512],
                            v_t[:, i, pair * 2 + u, :],
                            pend[i][:, u, lo:512],
                            start=(i == 0), stop=last,
                            skip_group_check=True)
                    del pend[i]

                pend = {}
                prev = None
                for i in range(nsb_j):
                    for fn in sched.get(i, ()):
                        fn()
                    r = i - 4 * j if causal else -1
                    lo = max(r, 0) * P
                    last = i == nsb_j - 1
                    scs = psA.tile([P, 2, 512], F32, tag="psA", name="scs")
                    pts = p_pool.tile([P, 2, 512], BF16, tag="pT", name="pts")
                    pend[i] = pts
                    for u in range(2):
                        dsl = slice(u * D, (u + 1) * D)
                        nc.tensor.matmul(
                            scs[:, u, lo:512],
                            kT[pair][dsl, i * P:(i + 1) * P],
                            qT[pair][dsl, j * 512 + lo:(j + 1) * 512],
                            start=True, stop=True)
                    if causal and r >= 0:
                        nc.vector.tensor_add(
                            scs[:, :, lo:lo + P], scs[:, :, lo:lo + P], mask)
                    nc.scalar.activation(
                        out=pts[:, :, lo:512], in_=scs[:, :, lo:512],
                        func=mybir.ActivationFunctionType.Exp, scale=SCALE)
                    if prev is not None:
                        emit_pv(*prev)
                    prev = (i, lo, last)
                if prev is not None:
                    emit_pv(*prev)
                for u in range(2):
                    # raw (unnormalized) head output + Z row gather
                    nc.vector.tensor_copy(
                        out=outcat[pair][u * D:(u + 1) * D,
                                         j * 512:(j + 1) * 512],
                        in_=outp[u][0:D, :])
                    k0, slot = _zslot(j, u)
                    nc.vector.tensor_copy(
                        out=zb[k0:k0 + 1, slot, :], in_=outp[u][D:D + 1, :])
            return zb

        rzbs = [None] * 4

        def recip_z(pair, zb, psl=slice(0, P), sl=slice(0, 3)):
            """Part A of normalization: 1/Z (DVE), f32->bf16.

            psl/sl select the zb region so pair 3 can normalize
            incrementally as its j-tiles finish without a dependency on
            later rows. Overlapping regions across calls rewrite the same
            values; the scheduler serializes them harmlessly.
            """
            if rzbs[pair] is None:
                rzbs[pair] = (
                    z_pool.tile([P, 3, 512], F32, tag="rz", name=f"rz{pair}"),
                    z_pool.tile([P, 3, 512], BF16, tag="rzb",
                                name=f"rzb{pair}"),
                )
            rz, rzb = rzbs[pair]
            nc.vector.reciprocal_approx_fast(
                out=rz[psl, sl, :], in_=zb[psl, sl, :])
            nc.vector.tensor_copy(out=rzb[psl, sl, :], in_=rz[psl, sl, :])

        def bcast_mul_1(pair, j, u):
            """Part B: broadcast 1/Z across partitions (K=1 matmul),
            scale outcat."""
            rzb = rzbs[pair][1]
            k0, slot = _zslot(j, u)
            bps = psQ.tile([P, 512], F32, tag="psQ", name="bps")
            nc.tensor.matmul(
                bps, ones_bc[k0:k0 + 1, :], rzb[k0:k0 + 1, slot, :],
                start=True, stop=True)
            osl = outcat[pair][u * D:(u + 1) * D, j * 512:(j + 1) * 512]
            nc.vector.tensor_mul(osl, osl, bps[u * D:(u + 1) * D, :])

        def bcast_mul(pair, js):
            for j in js:
                for u in range(2):
                    bcast_mul_1(pair, j, u)

        def yproj_ci(tc_, ci):
            """yT[c'-chunk, t-chunk] = sum_q wpt[q].T @ outcat[q]."""
            tg = slice(tc_ * 512, (tc_ + 1) * 512)
            yps = psQ.tile([P, 512], F32, tag="psQ", name="yps")
            for q in range(4):
                nc.tensor.matmul(
                    yps,
                    wpt_t[q][:, ci * P:(ci + 1) * P],
                    outcat[q][:, tg],
                    start=(q == 0), stop=(q == 3))
            yt = yst_pool.tile([P, 512], F32, tag="yst", name="yt")
            nc.scalar.activation(
                out=yt, in_=yps,
                func=mybir.ActivationFunctionType.Identity,
                bias=bpc[:, ci:ci + 1])
            nc.sync.dma_start(out=y_d[ci * P:(ci + 1) * P, tg], in_=yt)

        def yproj_chunk(tc_):
            for ci in range(NCH):
                yproj_ci(tc_, ci)

        def yproj_wide(tc0, ci):
            """Two t-chunks per PSUM tile: halves the identity/DMA count
            on the tail where nothing else is left to overlap."""
            yps = psA.tile([P, 2, 512], F32, tag="psA", name="ypsw")
            for w, tc_ in enumerate((tc0, tc0 + 1)):
                tg = slice(tc_ * 512, (tc_ + 1) * 512)
                for q in range(4):
                    nc.tensor.matmul(
                        yps[:, w, :],
                        wpt_t[q][:, ci * P:(ci + 1) * P],
                        outcat[q][:, tg],
                        start=(q == 0), stop=(q == 3))
            yt = yst_pool.tile([P, 2, 512], F32, tag="ystw", name="ytw")
            nc.scalar.activation(
                out=yt, in_=yps,
                func=mybir.ActivationFunctionType.Identity,
                bias=bpc[:, ci:ci + 1])
            nc.sync.dma_start(
                out=y_d[ci * P:(ci + 1) * P, tc0 * 512:(tc0 + 2) * 512],
                in_=yt.rearrange("p w t -> p (w t)"))

        # ---- schedule ----
        # The exp stream on ScalarE is the per-pair bottleneck; all other
        # PE work (same pair's next qk quarter, pair0's v s-blocks, the
        # previous pair's normalization, pair3's output projection) is
        # chopped into ~1-2us units dispensed evenly between s-blocks, so
        # the PE stays warm and ScalarE is never starved. v_s(s) units are
        # scheduled so slice s lands before the s-block that consumes it.
        zbs = [None] * 4
        if causal:
            qk_q(0, 0)

            def L(fn, *a):
                return lambda: fn(*a)

            def warm():
                # dependency-free PE activity: keeps the HAM clock-gate at
                # 2.4GHz through exp-bound stretches and dependency stalls
                # (~53ns; the next matmul self-loads its own weights anyway)
                nc.tensor.ldweights(weights=ones_bc)

            def make_units(pr):
                def units(j):
                    us = []
                    if pr == 0:
                        us += [L(v_s, s) for s in range(4 * j, 4 * j + 4)]
                    if j < 3:
                        us += [L(qk_half, pr, j + 1, 0),
                               L(qk_half, pr, j + 1, 1)]
                    elif pr < 3:
                        us += [L(qk_half, pr + 1, 0, 0),
                               L(qk_half, pr + 1, 0, 1)]
                    if pr > 0:
                        if j == 1:
                            us.append(L(recip_z, pr - 1, zbs[pr - 1]))
                        elif j == 2:
                            us += [L(bcast_mul_1, pr - 1, jj, u)
                                   for jj in range(NTT) for u in range(2)]
                    if pr == 3:
                        # incremental self-normalize + output projection
                        if j == 2:
                            us.append(L(recip_z, 3, zbs[3]))  # j0+j1 rows
                            us += [L(bcast_mul_1, 3, 0, u) for u in range(2)]
                        elif j == 3:
                            # j2 rows live at partition 32 only: the
                            # partition-sliced recip avoids waiting on j3
                            us.append(L(recip_z, 3, zbs[3], slice(32, 33),
                                        slice(1, 3)))
                            us += [L(bcast_mul_1, 3, jj, u)
                                   for jj in (1, 2) for u in range(2)]
                            us += [L(yproj_ci, 0, ci) for ci in range(NCH)]
                    return us
                return units

            for pr in range(4):
                attention(pr, tile_units=make_units(pr))
            recip_z(3, zbs[3], psl=slice(64, 65), sl=slice(0, 2))  # j3 rows
            for ci in range(NCH):
                yproj_wide(1, ci)  # t-chunks 1+2; recip3/j3 hides under it
            bcast_mul(3, (3,))
            yproj_chunk(3)
        else:
            # non-causal: every j reads all of kT/v, so project fully first
            for qq in range(4):
                qk_q(0, qq)
                v_q(qq)
            for pr in range(4):
                zbs[pr] = attention(pr)
                if pr < 3:
                    for qq in range(4):
                        qk_q(pr + 1, qq)
                if pr > 0:
                    recip_z(pr - 1, zbs[pr - 1])
                    bcast_mul(pr - 1, range(NTT))
            recip_z(3, zbs[3])
            bcast_mul(3, range(NTT))
            for tc_ in range(NTT):
                yproj_chunk(tc_)


_NC_CACHE = {}
LAST_RESULTS = None


def kernel(x, Wq, Wk, Wv, Wp, bp, is_masked, **_unused):
    global LAST_RESULTS
    from ml_dtypes import bfloat16

    x = np.asarray(x, np.float32)
    Wq = np.asarray(Wq, np.float32)
    Wk = np.asarray(Wk, np.float32)
    Wv = np.asarray(Wv, np.float32)
    Wp = np.asarray(Wp, np.float32)
    bp = np.asarray(bp, np.float32)
    causal = bool(np.asarray(is_masked).item())

    if causal not in _NC_CACHE:
        _NC_CACHE[causal] = _build(causal)
    nc = _NC_CACHE[causal]

    # host-side layout prep
    wq_r = np.ascontiguousarray(Wq.transpose(1, 0, 2).reshape(C, H * D))
    wk_r = np.ascontiguousarray(Wk.transpose(1, 0, 2).reshape(C, H * D))
    wv_r = np.ascontiguousarray(Wv.transpose(1, 0, 2).reshape(C, H * D))
    wpt = np.ascontiguousarray(Wp.T)
    zeros = np.zeros_like(bp)

    xTs = [np.ascontiguousarray(x[b].T).astype(bfloat16) for b in range(B)]
    in_maps = []
    for core in range(8):
        b, hh = core // 2, core % 2
        csl = slice(hh * HL * D, (hh + 1) * HL * D)
        in_maps.append({
            "xT": xTs[b],
            "wq": np.ascontiguousarray(wq_r[:, csl]).astype(bfloat16),
            "wk": np.ascontiguousarray(wk_r[:, csl]).astype(bfloat16),
            "wv": np.ascontiguousarray(wv_r[:, csl]).astype(bfloat16),
            "wpt": np.ascontiguousarray(wpt[csl, :]).astype(bfloat16),
            "bp": bp if hh == 0 else zeros,
        })

    trace = bool(int(os.environ.get("KERNEL_TRACE", "0")))
    res = run_bass_kernel_spmd(
        nc, in_maps, core_ids=list(range(8)), trace=trace)
    LAST_RESULTS = res

    y = np.empty((B, T, C), np.float32)
    for b in range(B):
        y[b] = res.results[2 * b]["y"].T + res.results[2 * b + 1]["y"].T
    return y
